# revision 24
# baseline (speedup 1.0000x reference)
"""Self-contained Trainium2 Bass kernel for one dense transformer block.

Problem: B=2, T=2048, C=1024, H=16 heads (D=64), MLP hidden 4096, causal
attention, exact gelu, fp32 I/O.

Sharding: pure data parallel across 8 cores, zero collectives.
Core c handles batch c//4, query block c%4 (512 rows). The host permutes
each core's batch tokens so its query rows land at positions 0..511 —
every core then runs the *identical* program; causality is carried by
per-core data (a per-key-row exp bias plus a fixed triangle for the four
diagonal chunks). K/V are (redundantly) computed for the whole batch on
every core.

On-device per core:
  A: LN1 -> PE-transpose h (LN gamma/beta fused into the PSUM
     evacuation) -> Q^T/K^T (head-pair packed, channels on partitions)
     and V (token-major, [V_h|1] augmented, spilled to DRAM per head)
  B: per head pair: S^T = K_h^T . Q_h (keys on partitions, the two
     heads' matmuls packed into disjoint PE row groups), exp on ACT with
     a per-key-row additive bias (-30000 masks whole rows), triangle
     multiply on DVE for the 4 diagonal chunks only, AV matmul with a
     ones-column giving the softmax denominator for free; normalize and
     assemble Y^T
  C: WO projection + residual, LN2, transpose (gamma/beta fused)
  D: MLP up (gelu fused into PSUM evacuation on ACT) + MLP down +
     residual
All matmuls run as float32r (full PE rate at N>=256).
"""
import sys
import os

sys.path.insert(0, "/opt/trn_rl_repo")

import numpy as np
from contextlib import ExitStack

import concourse.bass as bass
import concourse.tile as tile
from concourse import bacc, mybir
from concourse.masks import make_identity

F32 = mybir.dt.float32
F32R = mybir.dt.float32r
AF = mybir.ActivationFunctionType
OP = mybir.AluOpType

B, T, C, H, D, FF = 2, 2048, 1024, 16, 64, 4096
P = 128
NCORES = 8
EPS = 1e-5
CCH = C // P     # 8 channel chunks
TT = T // P      # 16 token tiles
QTOK = 512       # query rows per core
QT_TILES = QTOK // P  # 4
KC = T // P      # 16 key chunks
FFC = FF // P    # 32 ff chunks
SCALE = 1.0 / np.sqrt(D)
NEG = -30000.0


def _layer_norm_stats(nc, pool, x_ap):
    """mean/rstd of x_ap [128, 1024] along free dim -> (mv, rstd)."""
    stats = pool.tile([P, 2 * nc.vector.BN_STATS_DIM], F32, tag="ln_stats")
    xr = x_ap.rearrange("p (s f) -> p s f", s=2)
    for s in range(2):
        nc.vector.bn_stats(
            stats[:, s * nc.vector.BN_STATS_DIM:(s + 1) * nc.vector.BN_STATS_DIM],
            xr[:, s, :])
    mv = pool.tile([P, nc.vector.BN_AGGR_DIM], F32, tag="ln_mv")
    nc.vector.bn_aggr(mv[:], stats[:].rearrange("p (s f) -> p s f", s=2))
    eps_t = pool.tile([P, 1], F32, tag="ln_eps")
    nc.vector.memset(eps_t[:], EPS)
    rstd = pool.tile([P, 1], F32, tag="ln_rstd")
    nc.scalar.activation(rstd[:], mv[:, 1:2], AF.Sqrt, bias=eps_t[:], scale=1.0)
    nc.vector.reciprocal(rstd[:], rstd[:])
    return mv, rstd


def _bcast_load(nc, pool, dram_ap, n, tag):
    """Load a [n]-vector from DRAM broadcast across 128 partitions."""
    t = pool.tile([P, n], F32, tag=tag)
    src = bass.AP(tensor=dram_ap.tensor, offset=dram_ap.offset,
                  ap=[[0, P]] + [list(pair) for pair in dram_ap.ap])
    nc.sync.dma_start(t[:], src)
    return t


def _build_nc_inner(act_fn="gelu", phases="ABCD"):
    nc = bacc.Bacc("TRN2", target_bir_lowering=False, debug=False)

    x_d = nc.dram_tensor("x", (T, C), F32, kind="ExternalInput")
    mb_d = nc.dram_tensor("mbias", (KC, P), F32, kind="ExternalInput")
    wq_d = nc.dram_tensor("wq", (C, C), F32, kind="ExternalInput")
    wk_d = nc.dram_tensor("wk", (C, C), F32, kind="ExternalInput")
    wv_d = nc.dram_tensor("wv", (C, C), F32, kind="ExternalInput")
    wo_d = nc.dram_tensor("wo", (C, C), F32, kind="ExternalInput")
    bq_d = nc.dram_tensor("bq", (C,), F32, kind="ExternalInput")
    bk_d = nc.dram_tensor("bk", (C,), F32, kind="ExternalInput")
    bv_d = nc.dram_tensor("bv", (C,), F32, kind="ExternalInput")
    bo_d = nc.dram_tensor("bo", (C,), F32, kind="ExternalInput")
    ln1w_d = nc.dram_tensor("ln1_w", (C,), F32, kind="ExternalInput")
    ln1b_d = nc.dram_tensor("ln1_b", (C,), F32, kind="ExternalInput")
    ln2w_d = nc.dram_tensor("ln2_w", (C,), F32, kind="ExternalInput")
    ln2b_d = nc.dram_tensor("ln2_b", (C,), F32, kind="ExternalInput")
    w1_d = nc.dram_tensor("w1", (C, FF), F32, kind="ExternalInput")
    b1_d = nc.dram_tensor("b1", (FF,), F32, kind="ExternalInput")
    w2_d = nc.dram_tensor("w2", (FF, C), F32, kind="ExternalInput")
    b2_d = nc.dram_tensor("b2", (C,), F32, kind="ExternalInput")
    out_d = nc.dram_tensor("out", (QTOK, C), F32, kind="ExternalOutput")

    x_t = x_d[:].rearrange("(tt p) c -> tt p c", p=P)          # [16,128,1024]
    wq_r = wq_d[:].rearrange("(cc p) o -> p cc o", p=P)        # [128,8,1024]
    wk_r = wk_d[:].rearrange("(cc p) o -> p cc o", p=P)
    wv_r = wv_d[:].rearrange("(cc p) o -> cc p o", p=P)        # [8,128,1024]
    wo_r = wo_d[:].rearrange("(cc p) o -> cc p o", p=P)
    w1_r = w1_d[:].rearrange("(cc p) o -> cc p o", p=P)        # [8,128,4096]
    w2_r = w2_d[:].rearrange("(fc p) o -> fc p o", p=P)        # [32,128,1024]
    bq_r = bq_d[:].rearrange("(o p) -> p o", p=P)              # [128, 8]
    bk_r = bk_d[:].rearrange("(o p) -> p o", p=P)
    b1_r = b1_d[:].rearrange("(o p) -> p o", p=P)              # [128, 32]
    ln1w_r = ln1w_d[:].rearrange("(cc p) -> p cc", p=P)        # [128, 8]
    ln1b_r = ln1b_d[:].rearrange("(cc p) -> p cc", p=P)
    ln2w_r = ln2w_d[:].rearrange("(cc p) -> p cc", p=P)
    ln2b_r = ln2b_d[:].rearrange("(cc p) -> p cc", p=P)
    mb_r = mb_d[:].rearrange("kc p -> p kc")                   # [128, 16]

    gelu_af = AF.Gelu if act_fn == "gelu" else AF.Tanh

    with tile.TileContext(nc) as tc, ExitStack() as ctx:
        # ---------- persistent pools ----------
        persist = ctx.enter_context(tc.tile_pool(name="persist", bufs=1))
        dram = ctx.enter_context(tc.tile_pool(name="dram", bufs=1, space="DRAM"))

        ident = persist.tile([P, P], F32, tag="ident")
        make_identity(nc, ident)
        bq_sb = persist.tile([P, CCH], F32, tag="bq")
        nc.sync.dma_start(bq_sb[:], bq_r)
        bk_sb = persist.tile([P, CCH], F32, tag="bk")
        nc.sync.dma_start(bk_sb[:], bk_r)
        b1_sb = persist.tile([P, FFC], F32, tag="b1")
        nc.sync.dma_start(b1_sb[:], b1_r)
        ln1w_sb = persist.tile([P, CCH], F32, tag="ln1w")
        nc.sync.dma_start(ln1w_sb[:], ln1w_r)
        ln1b_sb = persist.tile([P, CCH], F32, tag="ln1b")
        nc.sync.dma_start(ln1b_sb[:], ln1b_r)
        ln2w_sb = persist.tile([P, CCH], F32, tag="ln2w")
        nc.sync.dma_start(ln2w_sb[:], ln2w_r)
        ln2b_sb = persist.tile([P, CCH], F32, tag="ln2b")
        nc.sync.dma_start(ln2b_sb[:], ln2b_r)
        mb_sb = persist.tile([P, KC], F32, tag="mb")
        nc.sync.dma_start(mb_sb[:], mb_r)

        # KT/QT live through phases A+B only (their pool closes after B).
        # YT (B..C) and h2T (C..D) share a small pool open for the whole
        # run; pools must close LIFO, so it opens before abp.
        bdp = ctx.enter_context(tc.tile_pool(name="bdp", bufs=1))
        es_ab = ExitStack()
        abp = es_ab.enter_context(tc.tile_pool(name="abp", bufs=1))
        KT_sb = abp.tile([P, CCH, T], F32R, tag="KT")          # 8 MB
        QT_sb = abp.tile([P, CCH, QTOK], F32R, tag="QT")       # 2 MB

        Vscr = dram.tile([KC, P, 16 * 65], F32)  # [V_h|1]x16 per kchunk
        x1scr = dram.tile([QT_TILES, P, C], F32)  # post-attn residual

        # =======================================================
        # Phase A: LN1, transpose, QKV projections (quarter-pipelined)
        # =======================================================
        with tc.tile_pool(name="phA", bufs=2) as pha, \
             tc.tile_pool(name="phA1", bufs=1) as pha1, \
             tc.tile_pool(name="xq4", bufs=1) as xqp, \
             tc.tile_pool(name="hTq", bufs=2) as htp, \
             tc.tile_pool(name="wvbp", bufs=2) as wvp, \
             tc.tile_pool(name="tp_ps", bufs=2, space="PSUM") as tpps:
            bv_bc = _bcast_load(nc, pha1, bv_d[:], C, "bv")
            wk_sb = pha1.tile([P, CCH, C], F32R, tag="wk_sb")   # 4 MB resident
            nc.sync.dma_start(wk_sb[:], wk_r.bitcast(F32R))

            for qtr in range(4):                # quarters of 512 tokens
                hTq = htp.tile([P, CCH, 512], F32R, tag="hTq", name="hTq")
                # ---- LN1 for the 4 token tiles of this quarter ----
                x_tiles = []
                for i in range(4):
                    gt = qtr * 4 + i
                    x_tile = xqp.tile([P, C], F32, tag=f"x{i}", name=f"x{i}")
                    nc.scalar.dma_start(x_tile[:], x_t[gt])
                    mv, rstd = _layer_norm_stats(nc, pha, x_tile[:])
                    nc.vector.tensor_scalar(x_tile[:], x_tile[:],
                                            scalar1=mv[:, 0:1],
                                            scalar2=rstd[:],
                                            op0=OP.subtract, op1=OP.mult)
                    x_tiles.append(x_tile)
                # ---- transposes (4 tiles per PSUM bank, one fused evac) ----
                for cc in range(CCH):
                    tp = tpps.tile([P, 512], F32, tag="tp")
                    for i in range(4):
                        nc.tensor.transpose(
                            tp[:, i * P:(i + 1) * P],
                            x_tiles[i][:, cc * P:(cc + 1) * P], ident[:])
                    nc.vector.tensor_scalar(
                        hTq[:, cc, :], tp[:],
                        scalar1=ln1w_sb[:, cc:cc + 1],
                        scalar2=ln1b_sb[:, cc:cc + 1],
                        op0=OP.mult, op1=OP.add)

                # ---- Q projection (quarter 0 holds all 512 queries) ----
                if qtr == 0:
                    with tc.tile_pool(name="q_ps", bufs=2, space="PSUM") as qps, \
                         tc.tile_pool(name="wqsp", bufs=1) as wqsp:
                        for occ in range(CCH):
                            qp = qps.tile([P, QTOK], F32, tag="qp")
                            for ch in range(2):
                                wqs = wqsp.tile([P, 4, P], F32R, tag="wqs",
                                                name="wqs")
                                nc.scalar.dma_start(
                                    wqs[:],
                                    wq_r[:, ch * 4:(ch + 1) * 4,
                                         occ * P:(occ + 1) * P].bitcast(F32R))
                                for c4 in range(4):
                                    cc = ch * 4 + c4
                                    nc.tensor.matmul(qp[:], wqs[:, c4, :],
                                                     hTq[:, cc, :],
                                                     start=(cc == 0),
                                                     stop=(cc == CCH - 1))
                            nc.vector.tensor_scalar_add(
                                QT_sb[:, occ, :], qp[:],
                                scalar1=bq_sb[:, occ:occ + 1])

                # ---- K projection for this quarter ----
                with tc.tile_pool(name="k_ps", bufs=2, space="PSUM") as kpsp:
                    for occ in range(CCH):
                        kp = kpsp.tile([P, 512], F32, tag="kp")
                        for cc in range(CCH):
                            nc.tensor.matmul(
                                kp[:], wk_sb[:, cc, occ * P:(occ + 1) * P],
                                hTq[:, cc, :],
                                start=(cc == 0), stop=(cc == CCH - 1))
                        nc.vector.tensor_scalar_add(
                            KT_sb[:, occ, qtr * 512:(qtr + 1) * 512], kp[:],
                            scalar1=bk_sb[:, occ:occ + 1])

                # ---- V projection for this quarter ----
                with tc.tile_pool(name="v_ps", bufs=1, space="PSUM") as vpsp:
                    for occ2 in range(2):       # output halves of 512
                        vps = [vpsp.tile([P, 512], F32, tag=f"vp{i}",
                                         name=f"vp{i}")
                               for i in range(4)]
                        for cc in range(CCH):
                            wvb = wvp.tile([P, 512], F32R, tag="wvb")
                            nc.scalar.dma_start(
                                wvb[:],
                                wv_r[cc, :, occ2 * 512:(occ2 + 1) * 512]
                                .bitcast(F32R))
                            for i in range(4):
                                nc.tensor.matmul(
                                    vps[i][:],
                                    hTq[:, cc, i * P:(i + 1) * P],
                                    wvb[:],
                                    start=(cc == 0), stop=(cc == CCH - 1))
                        for i in range(4):
                            kc = qtr * 4 + i    # key chunk == token tile
                            stage = pha.tile([P, 8, 65], F32, tag="vstage")
                            nc.vector.tensor_tensor(
                                stage[:, :, 0:64],
                                vps[i][:].rearrange("p (h e) -> p h e", e=64),
                                bv_bc[:, occ2 * 512:(occ2 + 1) * 512]
                                .rearrange("p (h e) -> p h e", e=64),
                                op=OP.add)
                            nc.vector.memset(stage[:, :, 64:65], 1.0)
                            nc.scalar.dma_start(
                                Vscr[kc, :, occ2 * 520:(occ2 + 1) * 520],
                                stage[:].rearrange("p h e -> p (h e)"))

        # =======================================================
        # Phase B: attention
        # =======================================================
        if "B" in phases:
            with tc.tile_pool(name="phB", bufs=4) as phb, \
                 tc.tile_pool(name="vrow", bufs=2) as vrp, \
                 tc.tile_pool(name="phB1", bufs=1) as phb1, \
                 tc.tile_pool(name="st_ps", bufs=2, space="PSUM") as stps, \
                 tc.tile_pool(name="av_ps", bufs=2, space="PSUM") as avps:
                YT_sb = bdp.tile([P, CCH, QTOK], F32R, tag="YT")   # 2 MB
                # duplicated triangle masks for the 4 diagonal chunks:
                # tri2[kp, kc, q] = 1 if (q mod 512) >= 128*kc + kp else 0
                tri2 = phb1.tile([P, 4, 2 * QTOK], F32R, tag="tri2")
                for kc in range(4):
                    for dup in range(2):
                        sl = tri2[:, kc, dup * QTOK:(dup + 1) * QTOK]
                        nc.gpsimd.memset(sl.bitcast(F32), 1.0)
                        nc.gpsimd.affine_select(
                            out=sl.bitcast(F32), in_=sl.bitcast(F32),
                            compare_op=OP.is_ge, fill=0.0, base=-128 * kc,
                            pattern=[[1, QTOK]], channel_multiplier=-1)

                for hh in range(H // 2):
                    Vr = vrp.tile([P, KC, 130], F32R, tag="Vr")
                    nc.sync.dma_start(
                        Vr[:], Vscr[:, :, 130 * hh:130 * hh + 130]
                        .rearrange("kc p e -> p kc e").bitcast(F32R))
                    Yp0 = avps.tile([P, QTOK], F32, tag="Yp0")
                    Yp1 = avps.tile([P, QTOK], F32, tag="Yp1")
                    for kc in range(KC):
                        # one 2-bank PSUM tile holds both heads' S^T
                        Sp = stps.tile([P, 2 * QTOK], F32, tag="Sp")
                        # the two heads' S^T matmuls use disjoint PE row
                        # groups (partitions 0:64 / 64:128) -> concurrent
                        nc.tensor.matmul(
                            Sp[:, 0:QTOK],
                            KT_sb[0:64, hh, kc * P:(kc + 1) * P],
                            QT_sb[0:64, hh, :], start=True, stop=True)
                        nc.tensor.matmul(
                            Sp[:, QTOK:2 * QTOK],
                            KT_sb[64:128, hh, kc * P:(kc + 1) * P],
                            QT_sb[64:128, hh, :], start=True, stop=True)
                        E = phb.tile([P, 2 * QTOK], F32R, tag="E")
                        nc.scalar.activation(E[:], Sp[:], AF.Exp,
                                             bias=mb_sb[:, kc:kc + 1],
                                             scale=float(SCALE))
                        if kc < 4:
                            nc.vector.tensor_tensor(E[:], E[:], tri2[:, kc, :],
                                                    op=OP.mult)
                        nc.tensor.matmul(Yp0[0:65, :],
                                         Vr[:, kc, 0:65], E[:, 0:QTOK],
                                         start=(kc == 0), stop=(kc == KC - 1))
                        nc.tensor.matmul(Yp1[0:65, :],
                                         Vr[:, kc, 65:130], E[:, QTOK:2 * QTOK],
                                         start=(kc == 0), stop=(kc == KC - 1))
                    for half, Yp in ((0, Yp0), (1, Yp1)):
                        rec = phb.tile([P, QTOK], F32, tag="rec")
                        nc.vector.reciprocal(rec[64:65, :], Yp[64:65, :])
                        rec0 = phb.tile([P, QTOK], F32, tag="rec0")
                        nc.sync.dma_start(rec0[0:1, :], rec[64:65, :])
                        recb = phb.tile([P, QTOK], F32, tag="recb")
                        nc.gpsimd.partition_broadcast(recb[0:64, :],
                                                      rec0[0:1, :])
                        stage = phb.tile([P, QTOK], F32R, tag="ystage")
                        nc.vector.tensor_tensor(stage[0:64, :], Yp[0:64, :],
                                                recb[0:64, :], op=OP.mult)
                        nc.sync.dma_start(
                            YT_sb[64 * half:64 * half + 64, hh, :],
                            stage[0:64, :])

            es_ab.close()   # KT/QT no longer needed

        if "B" not in phases:
            es_ab.close()

        # =======================================================
        # Phase C: WO projection + residual + LN2 + transpose
        # =======================================================
        if "C" in phases:
            with tc.tile_pool(name="phC", bufs=3) as phc, \
                 tc.tile_pool(name="phC1", bufs=1) as phc1, \
                 tc.tile_pool(name="wo_ps", bufs=2, space="PSUM") as wops, \
                 tc.tile_pool(name="tp2_ps", bufs=2, space="PSUM") as tp2ps:
                h2T = bdp.tile([P, CCH, QTOK], F32R, tag="h2T")    # 2 MB
                bo_bc = _bcast_load(nc, phc1, bo_d[:], C, "bo")
                WO_sb = phc1.tile([P, CCH, C], F32R, tag="WO")     # 4 MB
                for cc in range(CCH):
                    nc.sync.dma_start(WO_sb[:, cc, :], wo_r[cc].bitcast(F32R))
                for tt in range(QT_TILES):
                    xq_tile = phc.tile([P, C], F32, tag="xq")
                    nc.sync.dma_start(xq_tile[:], x_t[tt])
                    # xq += bo once (attn bias, same for every token)
                    nc.vector.tensor_tensor(xq_tile[:], xq_tile[:], bo_bc[:],
                                            op=OP.add)
                    x1_tile = phc.tile([P, C], F32, tag="x1t")
                    for oc2 in range(2):
                        wp = wops.tile([P, 512], F32, tag="wp")
                        for cc in range(CCH):
                            nc.tensor.matmul(
                                wp[:],
                                YT_sb[:, cc, tt * P:(tt + 1) * P],
                                WO_sb[:, cc, oc2 * 512:(oc2 + 1) * 512],
                                start=(cc == 0), stop=(cc == CCH - 1))
                        sl = slice(oc2 * 512, (oc2 + 1) * 512)
                        nc.vector.tensor_tensor(x1_tile[:, sl], wp[:],
                                                xq_tile[:, sl], op=OP.add)
                    nc.sync.dma_start(x1scr[tt], x1_tile[:])
                    mv, rstd = _layer_norm_stats(nc, phc, x1_tile[:])
                    h2_tile = phc.tile([P, C], F32, tag="h2")
                    nc.vector.tensor_scalar(h2_tile[:], x1_tile[:],
                                            scalar1=mv[:, 0:1], scalar2=rstd[:],
                                            op0=OP.subtract, op1=OP.mult)
                    for cc in range(CCH):
                        tp = tp2ps.tile([P, P], F32, tag="tp2")
                        nc.tensor.transpose(
                            tp[:], h2_tile[:, cc * P:(cc + 1) * P], ident[:])
                        nc.vector.tensor_scalar(
                            h2T[:, cc, tt * P:(tt + 1) * P], tp[:],
                            scalar1=ln2w_sb[:, cc:cc + 1],
                            scalar2=ln2b_sb[:, cc:cc + 1],
                            op0=OP.mult, op1=OP.add)


        # =======================================================
        # Phase D: MLP
        # =======================================================
        if "D" in phases:
            with tc.tile_pool(name="phD", bufs=3) as phd, \
                 tc.tile_pool(name="phD1", bufs=1) as phd1:
                upT = phd1.tile([P, FFC, QTOK], F32R, tag="upT")   # 8 MB
                with tc.tile_pool(name="up_ps", bufs=2, space="PSUM") as upps:
                    for fcg in range(FFC // 4):
                        ups = [upps.tile([P, QTOK], F32, tag=f"up{i}",
                                         name=f"up{i}")
                               for i in range(4)]
                        for cc in range(CCH):
                            w1s = phd.tile([P, 512], F32R, tag="w1s")
                            nc.scalar.dma_start(
                                w1s[:],
                                w1_r[cc, :, fcg * 512:(fcg + 1) * 512]
                                .bitcast(F32R))
                            for i in range(4):
                                nc.tensor.matmul(
                                    ups[i][:], w1s[:, i * P:(i + 1) * P],
                                    h2T[:, cc, :],
                                    start=(cc == 0), stop=(cc == CCH - 1))
                        for i in range(4):
                            fc = fcg * 4 + i
                            nc.scalar.activation(upT[:, fc, :], ups[i][:],
                                                 gelu_af,
                                                 bias=b1_sb[:, fc:fc + 1],
                                                 scale=1.0)

                b2_bc = _bcast_load(nc, phd1, b2_d[:], C, "b2")
                with tc.tile_pool(name="dn_ps", bufs=1, space="PSUM") as dnps:
                    dps = [dnps.tile([P, 512], F32, tag=f"dp{i}", name=f"dp{i}")
                           for i in range(8)]
                    for fc in range(FFC):
                        w2b = phd.tile([P, C], F32R, tag="w2b")
                        nc.scalar.dma_start(w2b[:], w2_r[fc].bitcast(F32R))
                        for tt in range(QT_TILES):
                            for oc2 in range(2):
                                nc.tensor.matmul(
                                    dps[tt * 2 + oc2][:],
                                    upT[:, fc, tt * P:(tt + 1) * P],
                                    w2b[:, oc2 * 512:(oc2 + 1) * 512],
                                    start=(fc == 0), stop=(fc == FFC - 1))
                    for tt in range(QT_TILES):
                        x1b = phd.tile([P, C], F32, tag="x1b")
                        nc.sync.dma_start(x1b[:], x1scr[tt])
                        nc.vector.tensor_tensor(x1b[:], x1b[:], b2_bc[:],
                                                op=OP.add)
                        for oc2 in range(2):
                            sl = slice(oc2 * 512, (oc2 + 1) * 512)
                            o_tile = phd.tile([P, 512], F32, tag="o_tile")
                            nc.vector.tensor_tensor(o_tile[:],
                                                    dps[tt * 2 + oc2][:],
                                                    x1b[:, sl], op=OP.add)
                            nc.sync.dma_start(
                                out_d[:].rearrange("(tt p) c -> tt p c",
                                                   p=P)[tt, :, sl],
                                o_tile[:])

    nc.finalize()
    return nc


def build_nc(act_fn="gelu", phases="ABCD"):
    return _build_nc_inner(act_fn, phases)


_NC_CACHE = {}


def _get_nc(act_fn="gelu"):
    if act_fn not in _NC_CACHE:
        _NC_CACHE[act_fn] = build_nc(act_fn)
    return _NC_CACHE[act_fn]


def _per_core_maps(inputs):
    """Build the 8 per-core input maps (permuted tokens + mask bias)."""
    x = np.asarray(inputs["x"], np.float32)          # [B, T, C]
    shared = {k: np.ascontiguousarray(np.asarray(v, np.float32))
              for k, v in inputs.items() if k != "x"}
    in_maps = []
    for c in range(NCORES):
        b, qb = divmod(c, 4)
        q_lo, q_hi = QTOK * qb, QTOK * qb + QTOK
        perm = np.concatenate([np.arange(q_lo, q_hi),
                               np.arange(0, q_lo),
                               np.arange(q_hi, T)]).astype(np.int64)
        x_c = np.ascontiguousarray(x[b][perm])
        # mbias[kc, kp]: 0 where the key row is fully allowed or in the
        # diagonal q-block (triangle handles it); NEG where the key comes
        # after every query.
        orig_k = perm                                # key position -> orig index
        mb = np.where(orig_k <= q_hi - 1, 0.0, NEG).astype(np.float32)
        in_maps.append({"x": x_c,
                        "mbias": np.ascontiguousarray(mb.reshape(KC, P)),
                        **shared})
    return in_maps


def kernel(**inputs):
    from concourse.bass_utils import run_bass_kernel_spmd
    nc = _get_nc("gelu")
    in_maps = _per_core_maps(inputs)
    trace = bool(int(os.environ.get("KERNEL_TRACE", "0")))
    res = run_bass_kernel_spmd(nc, in_maps, core_ids=list(range(NCORES)),
                               trace=trace)
    if trace and res.exec_time_ns is not None:
        print(f"HW exec time: {res.exec_time_ns} ns")
    kernel.last_results = res
    x = np.asarray(inputs["x"], np.float32)
    out = np.empty_like(x)
    for c in range(NCORES):
        b, qb = divmod(c, 4)
        out[b, QTOK * qb: QTOK * qb + QTOK] = res.results[c]["out"]
    return out
